# revision 1
# baseline (speedup 1.0000x reference)
"""Local (windowed) attention kernel for Trainium2, SPMD over 8 NeuronCores.

Problem (all shapes fixed):
  x [4, 4096, 1024] f32 -> qkv = x @ w_qkv; q,k,v = split(qkv)
  windows of 128 tokens attend to [prev window, own window] with a causal
  mask; NOTE the reference has a (faithful) bug: v2 = k2, so v is never
  used.  out = softmax(q k2^T / 32) @ k2 ; y = out @ w_out + b_out.

Sharding: data-parallel over (batch, seq-half): core c handles batch c//2,
tokens (c%2)*2048 ..+2048, with a 128-token key halo (zeros at the front of
a batch, matching the reference's zero pad of k).

Device algorithm per core (all matmuls bf16 with fp32 PSUM accumulate):
  qT = (w_q/32)^T @ xT            [1024, 2048]   (scale folded into w_q)
  kT = w_k^T @ xT                 [1024, 2176]   (incl. halo)
  z  = k @ w_out                  [2176, 1024]   (= values @ out-proj, since
                                                  v2 == k2 in the reference)
  per 128-token window w (16 of them):
    sim  = qT_w^T kT_w            PSUM [128, 256]
    L    = sim + mask             (DVE, reads PSUM)
    E,s  = exp(L), rowsum         (ACT with accum_out, E in bf16)
    ET   = PE-transpose(E)        [2x 128x128]
    yps  = ET^T @ z_w             PSUM [128, 1024] (unnormalized)
    y    = yps * (1/s) + b_out    (one fused DVE op), DMA out (f32)

The host passes x already transposed/casted so the kernel does no input
transposes.  Everything except xT chunks stays SBUF-resident.
"""

import numpy as np
import ml_dtypes

B, N, DIN, DINNER, DOUT, W = 4, 4096, 1024, 1024, 1024, 128
NCORES = 8
TPC = 2048                # main (query) tokens per core
TKT = TPC + W             # key tokens incl. halo = 2176
NWIN = TPC // W           # 16 windows per core
KD = DIN // 128           # 8 contraction tiles of 128
BF16 = ml_dtypes.bfloat16

# token chunks (in xT halo-inclusive coordinates) for the projection GEMMs
CHUNKS = [(0, 512), (512, 512), (1024, 512), (1536, 512), (2048, 128)]

_NC_CACHE = {}


def _build_nc():
    if "nc" in _NC_CACHE:
        return _NC_CACHE["nc"]

    import concourse.bacc as bacc
    import concourse.mybir as mybir
    import concourse.tile as tile
    from concourse.masks import make_identity

    f32 = mybir.dt.float32
    bf16 = mybir.dt.bfloat16

    nc = bacc.Bacc("TRN2", target_bir_lowering=False, debug=False)

    xT = nc.dram_tensor("xT", [DIN, TKT], bf16, kind="ExternalInput")
    wq = nc.dram_tensor("wq", [DIN, DINNER], bf16, kind="ExternalInput")
    wk = nc.dram_tensor("wk", [DIN, DINNER], bf16, kind="ExternalInput")
    wo = nc.dram_tensor("wo", [DINNER, DOUT], bf16, kind="ExternalInput")
    bias = nc.dram_tensor("bias", [128, DOUT], bf16, kind="ExternalInput")
    mask = nc.dram_tensor("mask", [W, 2 * W], f32, kind="ExternalInput")
    y = nc.dram_tensor("y", [TPC, DOUT], f32, kind="ExternalOutput")

    from contextlib import ExitStack

    with tile.TileContext(nc) as tc, ExitStack() as ctx:
        consts = ctx.enter_context(tc.tile_pool(name="consts", bufs=1))
        resid = ctx.enter_context(tc.tile_pool(name="resid", bufs=1))
        xin = ctx.enter_context(tc.tile_pool(name="xin", bufs=2))
        wwin = ctx.enter_context(tc.tile_pool(name="wwin", bufs=4))
        ystage = ctx.enter_context(tc.tile_pool(name="ystage", bufs=3))
        pmm = ctx.enter_context(tc.tile_pool(name="pmm", bufs=4, space="PSUM"))
        psim = ctx.enter_context(tc.tile_pool(name="psim", bufs=2, space="PSUM"))
        ptr = ctx.enter_context(tc.tile_pool(name="ptr", bufs=2, space="PSUM"))

        # ---- tiles ----------------------------------------------------------
        wq_sb = consts.tile([128, KD, DINNER], bf16)
        wk_sb = consts.tile([128, KD, DINNER], bf16)
        wo_sb = consts.tile([128, KD, DOUT], bf16)
        bias_sb = consts.tile([128, DOUT], bf16)
        mask_sb = consts.tile([W, 2 * W], f32)
        ident = consts.tile([128, 128], bf16)

        qT_sb = resid.tile([128, KD, TPC], bf16)
        kT_sb = resid.tile([128, KD, TKT], bf16)
        z_sb = resid.tile([128, TKT // 128, DOUT], bf16)

        # PE is data-starved for the first ~10us (DMA init + first chunk
        # arrival) and HAM holds it at half clock for its first ~3.4us of
        # sustained work.  Burn the idle window on dummy matmuls over a
        # memset tile so the clock gate opens before real data lands.
        warm = consts.tile([128, 512], bf16)
        nc.gpsimd.memset(warm[:], 0.0)
        wps = pmm.tile([128, 512], f32, tag="mm")
        for i in range(8):
            nc.tensor.matmul(
                wps[:], warm[:, 0:128], warm[:], start=(i == 0), stop=(i == 7)
            )

        # ---- phase 1: qT / kT projections, streaming xT chunks -------------
        # DMA order is chosen so the first kT matmul is gated only on wk +
        # the first xt chunk (~3MB), not the whole 10.5MB input set.  wq
        # arrives while kT chunk 0 computes; wo/bias/mask arrive during the
        # remaining projection chunks.
        # one dma_start per tensor via multi-dim APs (issue overhead on the
        # sync sequencer is ~0.5us per dma_start); split a small k=0 head off
        # wk / first xt chunk so the very first matmul gates on ~0.4MB only.
        wk_r = wk.rearrange("(k p) n -> p k n", p=128)
        wq_r = wq.rearrange("(k p) n -> p k n", p=128)
        wo_r = wo.rearrange("(k p) n -> p k n", p=128)
        xT_r = xT.rearrange("(k p) n -> p k n", p=128)
        for ci, (c0, cn) in enumerate(CHUNKS):
            xt = xin.tile([128, KD, 512], bf16, tag="xt")
            if ci == 0:
                # per-k transfers, interleaved so the k-th matmul group
                # chases the k-th (wk, xt) arrival pair (~1.1us apart)
                # instead of waiting for one 2.6MB transfer
                nc.sync.dma_start(wk_sb[:, 0, 0:128], wk_r[:, 0, 0:128])
                for k in range(KD):
                    if k == 0:
                        nc.sync.dma_start(wk_sb[:, 0, 128:], wk_r[:, 0, 128:])
                    else:
                        nc.sync.dma_start(wk_sb[:, k, :], wk_r[:, k, :])
                    nc.sync.dma_start(xt[:, k, :cn], xT_r[:, k, c0 : c0 + cn])
            else:
                nc.sync.dma_start(xt[:, :, :cn], xT_r[:, :, c0 : c0 + cn])
            # kT over the full halo-inclusive range
            for m in range(KD):
                ps = pmm.tile([128, 512], f32, tag="mm")
                for k in range(KD):
                    nc.tensor.matmul(
                        ps[:, :cn],
                        wk_sb[:, k, 128 * m : 128 * (m + 1)],
                        xt[:, k, :cn],
                        start=(k == 0),
                        stop=(k == KD - 1),
                    )
                nc.vector.tensor_copy(kT_sb[:, m, c0 : c0 + cn], ps[:, :cn])
            if ci == 0:
                # issued after chunk-0 kT matmuls: overlaps with that compute
                nc.sync.dma_start(wq_sb[:], wq_r[:])
            # qT only over main tokens (xT cols >= W)
            q0 = max(c0, W)
            qn = c0 + cn - q0
            if qn > 0:
                xoff = q0 - c0
                for m in range(KD):
                    ps = pmm.tile([128, 512], f32, tag="mm")
                    for k in range(KD):
                        nc.tensor.matmul(
                            ps[:, :qn],
                            wq_sb[:, k, 128 * m : 128 * (m + 1)],
                            xt[:, k, xoff : xoff + qn],
                            start=(k == 0),
                            stop=(k == KD - 1),
                        )
                    nc.vector.tensor_copy(
                        qT_sb[:, m, q0 - W : q0 - W + qn], ps[:, :qn]
                    )
            if ci == 0:
                nc.sync.dma_start(wo_sb[:], wo_r[:])
                nc.sync.dma_start(bias_sb[:], bias[:])
                nc.sync.dma_start(mask_sb[:], mask[:])
                make_identity(nc, ident)

        # ---- phase 2: z = k @ w_out  (token-major) -------------------------
        for t in range(TKT // 128):
            for nh in range(2):
                ps = pmm.tile([128, 512], f32, tag="mm")
                for k in range(KD):
                    nc.tensor.matmul(
                        ps[:],
                        kT_sb[:, k, 128 * t : 128 * (t + 1)],
                        wo_sb[:, k, 512 * nh : 512 * (nh + 1)],
                        start=(k == 0),
                        stop=(k == KD - 1),
                    )
                nc.scalar.copy(z_sb[:, t, 512 * nh : 512 * (nh + 1)], ps[:])

        # ---- phase 3: windows ----------------------------------------------
        for w in range(NWIN):
            sim = psim.tile([128, 2 * W], f32, tag="sim")
            for k in range(KD):
                nc.tensor.matmul(
                    sim[:],
                    qT_sb[:, k, W * w : W * (w + 1)],
                    kT_sb[:, k, W * w : W * (w + 2)],
                    start=(k == 0),
                    stop=(k == KD - 1),
                )
            L = wwin.tile([128, 2 * W], f32, tag="L")
            nc.vector.tensor_tensor(L[:], sim[:], mask_sb[:], op=_alu().add)
            E = wwin.tile([128, 2 * W], bf16, tag="E")
            s = wwin.tile([128, 1], f32, tag="s")
            nc.scalar.activation(
                E[:], L[:], _act().Exp, accum_out=s[:]
            )
            r = wwin.tile([128, 1], f32, tag="r")
            nc.vector.reciprocal(r[:], s[:])
            # transpose E -> ET [j, i] (two 128x128 blocks)
            et_ps = ptr.tile([128, 2, 128], bf16, tag="tr")
            nc.tensor.transpose(et_ps[:, 0, :], E[:, 0:128], ident[:])
            nc.tensor.transpose(et_ps[:, 1, :], E[:, 128:256], ident[:])
            ET = wwin.tile([128, 2, 128], bf16, tag="ET")
            nc.vector.tensor_copy(ET[:], et_ps[:])
            # y_w = ET^T @ z[w:w+2], normalized + bias on eviction
            yt = ystage.tile([128, DOUT], f32, tag="y")
            for nh in range(2):
                ps = pmm.tile([128, 512], f32, tag="mm")
                for jt in range(2):
                    nc.tensor.matmul(
                        ps[:],
                        ET[:, jt, :],
                        z_sb[:, w + jt, 512 * nh : 512 * (nh + 1)],
                        start=(jt == 0),
                        stop=(jt == 1),
                    )
                nc.vector.scalar_tensor_tensor(
                    yt[:, 512 * nh : 512 * (nh + 1)],
                    ps[:],
                    r[:],
                    bias_sb[:, 512 * nh : 512 * (nh + 1)],
                    op0=_alu().mult,
                    op1=_alu().add,
                )
            nc.sync.dma_start(y[W * w : W * (w + 1), :], yt[:])

    nc.compile()
    _NC_CACHE["nc"] = nc
    return nc


def _alu():
    import concourse.mybir as mybir

    return mybir.AluOpType


def _act():
    import concourse.mybir as mybir

    return mybir.ActivationFunctionType


def _make_mask():
    # row i (query), col j of [prev, cur]: masked (set very negative)
    # where j > i + W  (strictly causal within the 2-window lookback)
    i = np.arange(W)[:, None]
    j = np.arange(2 * W)[None, :]
    return np.where(j > i + W, np.float32(-1e30), np.float32(0.0))


def prep_in_maps(x, w_qkv, w_out, b_out):
    scale = np.float32(DINNER) ** np.float32(-0.5)
    wq = (w_qkv[:, :DINNER] * scale).astype(BF16)
    wk = np.ascontiguousarray(w_qkv[:, DINNER : 2 * DINNER]).astype(BF16)
    wo = w_out.astype(BF16)
    bias = np.broadcast_to(b_out.astype(BF16), (128, DOUT)).copy()
    mask = _make_mask()
    in_maps = []
    for c in range(NCORES):
        b, h = divmod(c, 2)
        xTc = np.zeros((DIN, TKT), dtype=BF16)
        xb = np.ascontiguousarray(x[b].T)  # [DIN, N]
        xTc[:, W:] = xb[:, h * TPC : (h + 1) * TPC].astype(BF16)
        if h == 1:
            xTc[:, :W] = xb[:, TPC - W : TPC].astype(BF16)
        in_maps.append(
            {"xT": xTc, "wq": wq, "wk": wk, "wo": wo, "bias": bias, "mask": mask}
        )
    return in_maps


def kernel(x, w_qkv, w_out, b_out, _trace=False):
    from concourse import bass_utils

    x = np.asarray(x)
    w_qkv = np.asarray(w_qkv)
    w_out = np.asarray(w_out)
    b_out = np.asarray(b_out)

    nc = _build_nc()
    in_maps = prep_in_maps(x, w_qkv, w_out, b_out)
    res = bass_utils.run_bass_kernel_spmd(
        nc, in_maps, core_ids=list(range(NCORES)), trace=_trace
    )
    out = np.empty((B, N, DOUT), dtype=np.float32)
    for c in range(NCORES):
        b, h = divmod(c, 2)
        out[b, h * TPC : (h + 1) * TPC, :] = res.results[c]["y"]
    if _trace:
        kernel.last_exec_time_ns = res.exec_time_ns
        kernel.last_results = res
    return out



# revision 2
# speedup vs baseline: 1.3741x; 1.3741x over previous
"""Local (windowed) attention kernel for Trainium2, SPMD over 8 NeuronCores.

Problem (all shapes fixed):
  x [4, 4096, 1024] f32 -> qkv = x @ w_qkv; q,k,v = split(qkv)
  windows of 128 tokens attend to [prev window, own window] with a causal
  mask; NOTE the reference has a (faithful) bug: v2 = k2, so v is never
  used.  out = softmax(q k2^T / 32) @ k2 ; y = out @ w_out + b_out.

Sharding: data-parallel over (batch, seq-half): core c handles batch c//2,
tokens (c%2)*2048 ..+2048, with a 128-token halo (zeros at the front of a
batch, matching the reference's zero pad of k).

Key algebraic refactor (saves 1.48x matmul FLOPs vs projecting q,k):
  sim = q k^T / 32 = x (Wq Wk^T / 32) x^T = (x @ Wqk) x^T
  z   = k @ w_out  = x (Wk @ Wo)          =  x @ Wkv
with Wqk, Wkv precomputed on the host in f32.  The k projection vanishes;
x itself (SBUF-resident) serves as the sim moving operand and the z
stationary operand.

Device algorithm per core (all matmuls bf16 with fp32 PSUM accumulate):
  q'T = Wqk^T @ xT                  [1024, 2048]
  per 128-token tile t (17 incl. halo):
    z_t = xT_t^T @ Wkv              [128, 1024] token-major
  per 128-token window w (16), interleaved with the z tiles so PE never
  waits on the softmax chain:
    sim  = q'T_w^T xT_[w,w+2)      PSUM [128, 256]
    L    = sim + mask               (DVE, reads PSUM)
    E,s  = exp(L), rowsum           (ACT with accum_out, E in bf16)
    ET   = PE-transpose(E)          [2x 128x128]
    yps  = ET^T @ z_[w,w+2)        PSUM [128, 1024] (unnormalized)
    y    = yps * (1/s) + b_out      (one fused DVE op), DMA out (f32)
"""

import numpy as np
import ml_dtypes

B, N, DIN, DINNER, DOUT, W = 4, 4096, 1024, 1024, 1024, 128
NCORES = 8
TPC = 2048                # main (query) tokens per core
TKT = TPC + W             # tokens incl. halo = 2176
NWIN = TPC // W           # 16 windows per core
NT = TKT // 128           # 17 token tiles incl. halo
KD = DIN // 128           # 8 contraction tiles of 128
BF16 = ml_dtypes.bfloat16

# q' chunks in xT halo-inclusive columns (q tokens are cols 128..2176)
QCHUNKS = [(128, 512), (640, 512), (1152, 512), (1664, 512)]

_NC_CACHE = {}


def _build_nc():
    if "nc" in _NC_CACHE:
        return _NC_CACHE["nc"]

    import concourse.bacc as bacc
    import concourse.mybir as mybir
    import concourse.tile as tile
    from concourse.masks import make_identity

    f32 = mybir.dt.float32
    bf16 = mybir.dt.bfloat16

    nc = bacc.Bacc("TRN2", target_bir_lowering=False, debug=False)

    xT = nc.dram_tensor("xT", [DIN, TKT], bf16, kind="ExternalInput")
    wqk = nc.dram_tensor("wqk", [DIN, DINNER], bf16, kind="ExternalInput")
    wkv = nc.dram_tensor("wkv", [DIN, DOUT], bf16, kind="ExternalInput")
    bias = nc.dram_tensor("bias", [128, DOUT], bf16, kind="ExternalInput")
    mask = nc.dram_tensor("mask", [W, 2 * W], f32, kind="ExternalInput")
    y = nc.dram_tensor("y", [TPC, DOUT], f32, kind="ExternalOutput")

    from contextlib import ExitStack

    with tile.TileContext(nc) as tc, ExitStack() as ctx:
        consts = ctx.enter_context(tc.tile_pool(name="consts", bufs=1))
        resid = ctx.enter_context(tc.tile_pool(name="resid", bufs=1))
        wwin = ctx.enter_context(tc.tile_pool(name="wwin", bufs=4))
        ystage = ctx.enter_context(tc.tile_pool(name="ystage", bufs=3))
        pmm = ctx.enter_context(tc.tile_pool(name="pmm", bufs=4, space="PSUM"))
        psim = ctx.enter_context(tc.tile_pool(name="psim", bufs=2, space="PSUM"))
        ptr = ctx.enter_context(tc.tile_pool(name="ptr", bufs=2, space="PSUM"))

        # ---- tiles ----------------------------------------------------------
        wqk_sb = consts.tile([128, KD, DINNER], bf16)
        wkv_sb = consts.tile([128, KD, DOUT], bf16)
        bias_sb = consts.tile([128, DOUT], bf16)
        mask_sb = consts.tile([W, 2 * W], f32)
        ident = consts.tile([128, 128], bf16)

        xT_sb = resid.tile([128, KD, TKT], bf16)
        qT_sb = resid.tile([128, KD, TPC], bf16)
        z_sb = resid.tile([128, NT, DOUT], bf16)

        # PE is data-starved for the first ~8us (DMA init + first chunk
        # arrival) and HAM holds it at half clock for its first ~3.4us of
        # sustained work.  Burn the idle window on dummy matmuls over a
        # memset tile so the clock gate opens before real data lands.
        warm = consts.tile([128, 512], bf16)
        nc.gpsimd.memset(warm[:], 0.0)
        wps = pmm.tile([128, 512], f32, tag="mm")
        for i in range(8):
            nc.tensor.matmul(
                wps[:], warm[:, 0:128], warm[:], start=(i == 0), stop=(i == 7)
            )

        # ---- DMAs -----------------------------------------------------------
        # The first q' matmul gates on wqk k=0 head + xT k=0 chunk-0 cols;
        # later k groups chase their (wqk_k, xT_k) arrival pairs.  The rest
        # of xT, then wkv/bias/mask, land under the q' chunk compute.
        wqk_r = wqk.rearrange("(k p) n -> p k n", p=128)
        wkv_r = wkv.rearrange("(k p) n -> p k n", p=128)
        xT_r = xT.rearrange("(k p) n -> p k n", p=128)
        C0 = 640  # xT head columns: halo + q' chunk 0
        nc.sync.dma_start(wqk_sb[:, 0, 0:128], wqk_r[:, 0, 0:128])
        for k in range(KD):
            if k == 0:
                nc.sync.dma_start(wqk_sb[:, 0, 128:], wqk_r[:, 0, 128:])
            else:
                nc.sync.dma_start(wqk_sb[:, k, :], wqk_r[:, k, :])
            nc.sync.dma_start(xT_sb[:, k, 0:C0], xT_r[:, k, 0:C0])

        # ---- phase 1: q'T projection ---------------------------------------
        for ci, (c0, cn) in enumerate(QCHUNKS):
            for m in range(KD):
                ps = pmm.tile([128, 512], f32, tag="mm")
                for k in range(KD):
                    nc.tensor.matmul(
                        ps[:, :cn],
                        wqk_sb[:, k, 128 * m : 128 * (m + 1)],
                        xT_sb[:, k, c0 : c0 + cn],
                        start=(k == 0),
                        stop=(k == KD - 1),
                    )
                nc.vector.tensor_copy(qT_sb[:, m, c0 - W : c0 - W + cn], ps[:, :cn])
            if ci == 0:
                # issued after chunk-0 matmuls: overlaps with that compute
                nc.sync.dma_start(xT_sb[:, :, C0:], xT_r[:, :, C0:])
            elif ci == 1:
                nc.sync.dma_start(wkv_sb[:], wkv_r[:])
                nc.sync.dma_start(bias_sb[:], bias[:])
                nc.sync.dma_start(mask_sb[:], mask[:])
                make_identity(nc, ident)

        # ---- phase 2: z tiles interleaved with attention windows -----------
        def z_tile(t):
            for nh in range(2):
                ps = pmm.tile([128, 512], f32, tag="mm")
                for k in range(KD):
                    nc.tensor.matmul(
                        ps[:],
                        xT_sb[:, k, 128 * t : 128 * (t + 1)],
                        wkv_sb[:, k, 512 * nh : 512 * (nh + 1)],
                        start=(k == 0),
                        stop=(k == KD - 1),
                    )
                nc.scalar.copy(z_sb[:, t, 512 * nh : 512 * (nh + 1)], ps[:])

        z_tile(0)
        z_tile(1)
        for w in range(NWIN):
            sim = psim.tile([128, 2 * W], f32, tag="sim")
            for k in range(KD):
                nc.tensor.matmul(
                    sim[:],
                    qT_sb[:, k, W * w : W * (w + 1)],
                    xT_sb[:, k, W * w : W * (w + 2)],
                    start=(k == 0),
                    stop=(k == KD - 1),
                )
            L = wwin.tile([128, 2 * W], f32, tag="L")
            nc.vector.tensor_tensor(L[:], sim[:], mask_sb[:], op=_alu().add)
            E = wwin.tile([128, 2 * W], bf16, tag="E")
            s = wwin.tile([128, 1], f32, tag="s")
            nc.scalar.activation(E[:], L[:], _act().Exp, accum_out=s[:])
            r = wwin.tile([128, 1], f32, tag="r")
            nc.vector.reciprocal(r[:], s[:])
            # the next z tile here keeps PE busy while DVE/ACT produce E
            if w + 2 < NT:
                z_tile(w + 2)
            # transpose E -> ET [j, i] (two 128x128 blocks)
            et_ps = ptr.tile([128, 2, 128], bf16, tag="tr")
            nc.tensor.transpose(et_ps[:, 0, :], E[:, 0:128], ident[:])
            nc.tensor.transpose(et_ps[:, 1, :], E[:, 128:256], ident[:])
            ET = wwin.tile([128, 2, 128], bf16, tag="ET")
            nc.vector.tensor_copy(ET[:], et_ps[:])
            # y_w = ET^T @ z[w:w+2], normalized + bias on eviction
            yt = ystage.tile([128, DOUT], f32, tag="y")
            for nh in range(2):
                ps = pmm.tile([128, 512], f32, tag="mm")
                for jt in range(2):
                    nc.tensor.matmul(
                        ps[:],
                        ET[:, jt, :],
                        z_sb[:, w + jt, 512 * nh : 512 * (nh + 1)],
                        start=(jt == 0),
                        stop=(jt == 1),
                    )
                nc.vector.scalar_tensor_tensor(
                    yt[:, 512 * nh : 512 * (nh + 1)],
                    ps[:],
                    r[:],
                    bias_sb[:, 512 * nh : 512 * (nh + 1)],
                    op0=_alu().mult,
                    op1=_alu().add,
                )
            nc.sync.dma_start(y[W * w : W * (w + 1), :], yt[:])

    nc.compile()
    _NC_CACHE["nc"] = nc
    return nc


def _alu():
    import concourse.mybir as mybir

    return mybir.AluOpType


def _act():
    import concourse.mybir as mybir

    return mybir.ActivationFunctionType


def _make_mask():
    # row i (query), col j of [prev, cur]: masked (set very negative)
    # where j > i + W  (strictly causal within the 2-window lookback)
    i = np.arange(W)[:, None]
    j = np.arange(2 * W)[None, :]
    return np.where(j > i + W, np.float32(-1e30), np.float32(0.0))


def prep_in_maps(x, w_qkv, w_out, b_out):
    scale = np.float32(DINNER) ** np.float32(-0.5)
    wq = np.asarray(w_qkv[:, :DINNER], dtype=np.float32)
    wk = np.asarray(w_qkv[:, DINNER : 2 * DINNER], dtype=np.float32)
    wo = np.asarray(w_out, dtype=np.float32)
    wqk = ((wq @ wk.T) * scale).astype(BF16)
    wkv = (wk @ wo).astype(BF16)
    bias = np.broadcast_to(b_out.astype(BF16), (128, DOUT)).copy()
    mask = _make_mask()
    in_maps = []
    for c in range(NCORES):
        b, h = divmod(c, 2)
        xTc = np.zeros((DIN, TKT), dtype=BF16)
        xb = np.ascontiguousarray(x[b].T)  # [DIN, N]
        xTc[:, W:] = xb[:, h * TPC : (h + 1) * TPC].astype(BF16)
        if h == 1:
            xTc[:, :W] = xb[:, TPC - W : TPC].astype(BF16)
        in_maps.append(
            {"xT": xTc, "wqk": wqk, "wkv": wkv, "bias": bias, "mask": mask}
        )
    return in_maps


def kernel(x, w_qkv, w_out, b_out, _trace=False):
    from concourse import bass_utils

    x = np.asarray(x)
    w_qkv = np.asarray(w_qkv)
    w_out = np.asarray(w_out)
    b_out = np.asarray(b_out)

    nc = _build_nc()
    in_maps = prep_in_maps(x, w_qkv, w_out, b_out)
    res = bass_utils.run_bass_kernel_spmd(
        nc, in_maps, core_ids=list(range(NCORES)), trace=_trace
    )
    out = np.empty((B, N, DOUT), dtype=np.float32)
    for c in range(NCORES):
        b, h = divmod(c, 2)
        out[b, h * TPC : (h + 1) * TPC, :] = res.results[c]["y"]
    if _trace:
        kernel.last_exec_time_ns = res.exec_time_ns
        kernel.last_results = res
    return out
